# revision 4
# baseline (speedup 1.0000x reference)
"""AttnDecoderRNN single-step kernel for 8 Trainium2 NeuronCores.

Sharding (tensor-parallel, per spec hint):
  - attention (tiny) replicated on every core
  - comb_w output-sharded (256 rows/core)  -> AllGather x (1KB/core)
  - w_ih/w_hh output-sharded (768 rows/core, r/z/n aligned) -> AllGather h_new
  - out_w vocab-row-sharded (6283 rows/core, core 7 overlaps core 6)
  - emb: only the one indexed row is needed; gathered on host (8KB)

All matvecs run as DVE/GpSimd elementwise multiply (weight rows on
partitions, activation row broadcast along the free dim) + a reduction
pass on the Scalar engine (activation Identity with accum_out) or DVE
reduce_sum — no weight transposes, no TensorE (whose cold-clock tiny
matmuls measured ~10x slower than the same work on DVE). The kernel is
HBM-bandwidth-bound on the out_w stream.
"""

import numpy as np

H = 2048
V = 50257
L = 20
NC = 8
HS = H // NC          # 256 hidden rows per core
VS = 6283             # out_w rows per core (core 7 re-computes 7 rows)
VT = (VS + 127) // 128  # 50 vocab tiles per core
VPAD = VT * 128       # 6400

TRACE = False
LAST_RESULT = None

_NC_CACHE = {}


def _build():
    import concourse.bass as bass
    import concourse.bacc as bacc
    import concourse.tile as tile
    from concourse import mybir

    f32 = mybir.dt.float32
    AF = mybir.ActivationFunctionType
    OP = mybir.AluOpType
    RG = [list(range(NC))]
    X = mybir.AxisListType.X

    nc = bacc.Bacc("TRN2", target_bir_lowering=False, debug=False, num_devices=NC)

    # ---- I/O ----
    erow = nc.dram_tensor("erow", [1, H], f32, kind="ExternalInput")
    aw = nc.dram_tensor("aw", [L, 2 * H], f32, kind="ExternalInput")
    ab = nc.dram_tensor("ab", [1, L], f32, kind="ExternalInput")
    encT = nc.dram_tensor("encT", [H, L], f32, kind="ExternalInput")
    cw = nc.dram_tensor("cw", [HS, 2 * H], f32, kind="ExternalInput")
    cb = nc.dram_tensor("cb", [HS], f32, kind="ExternalInput")
    wg = {
        nm: nc.dram_tensor(nm, [HS, H], f32, kind="ExternalInput")
        for nm in ("wihr", "wihz", "wihn", "whhr", "whhz", "whhn")
    }
    bg = {
        nm: nc.dram_tensor(nm, [HS], f32, kind="ExternalInput")
        for nm in ("bihr", "bihz", "bihn", "bhhr", "bhhz", "bhhn")
    }
    h0row = nc.dram_tensor("h0row", [1, H], f32, kind="ExternalInput")
    h0k = nc.dram_tensor("h0k", [HS], f32, kind="ExternalInput")
    ow = nc.dram_tensor("ow", [VS, H], f32, kind="ExternalInput")
    ob = nc.dram_tensor("ob", [VPAD], f32, kind="ExternalInput")

    logits_o = nc.dram_tensor("logits", [128, VT], f32, kind="ExternalOutput")
    hnew_o = nc.dram_tensor("hnew", [1, H], f32, kind="ExternalOutput")
    attn_o = nc.dram_tensor("attn", [1, L], f32, kind="ExternalOutput")

    # internal DRAM: collective bounces + broadcast staging
    cc_in1 = nc.dram_tensor("cc_in1", [HS], f32)
    cc_out1 = nc.dram_tensor("cc_out1", [H], f32)
    cc_in2 = nc.dram_tensor("cc_in2", [HS], f32)
    cc_out2 = nc.dram_tensor("cc_out2", [H], f32)
    areg = nc.dram_tensor("areg", [L], f32)
    creg = nc.dram_tensor("creg", [H], f32)

    def bc(handle, n, cnt):
        # broadcast a contiguous DRAM range across n partitions
        a = handle.ap()
        return bass.AP(tensor=a.tensor, offset=a.offset, ap=[[0, n], [1, cnt]])

    with tile.TileContext(nc) as tc:
        with (
            tc.tile_pool(name="singles", bufs=1) as sg,
            tc.tile_pool(name="gw", bufs=4) as gwp,
            tc.tile_pool(name="lw", bufs=5) as lwp,
            tc.tile_pool(name="scr", bufs=3) as scrp,
        ):
            # broadcast rows used by several phases
            eb = sg.tile([128, H], f32)        # embedded row, all partitions
            nc.sync.dma_start(out=eb, in_=bc(erow, 128, H))
            h0b = sg.tile([128, H], f32)       # h0 row, all partitions
            nc.sync.dma_start(out=h0b, in_=bc(h0row, 128, H))

            # ---------- attention scores: aw @ [emb; h0] ----------
            # rows of attn_w on partitions 0..19, contraction along free dim
            adots = sg.tile([L, 2], f32)
            for hh in range(2):
                awh = gwp.tile([L, H], f32, name=f"awh{hh}", tag="gw")
                nc.sync.dma_start(out=awh, in_=aw[:, hh * H : (hh + 1) * H])
                s_t = scrp.tile([L, H], f32, name=f"ascr{hh}", tag="scr")
                nc.vector.tensor_mul(s_t, awh, (eb if hh == 0 else h0b)[:L, :])
                nc.scalar.activation(s_t, s_t, AF.Identity,
                                     accum_out=adots[:, hh : hh + 1])
            acol = sg.tile([L, 1], f32)
            nc.vector.tensor_add(acol, adots[:, 0:1], adots[:, 1:2])
            # to one row + bias + softmax
            arow = sg.tile([1, L], f32)
            nc.sync.dma_start(out=arow, in_=acol)
            ab_sb = sg.tile([1, L], f32)
            nc.sync.dma_start(out=ab_sb, in_=ab[:, :])
            nc.vector.tensor_add(arow, arow, ab_sb)
            amax = sg.tile([1, 1], f32)
            nc.vector.reduce_max(amax, arow, axis=X)
            negm = sg.tile([1, 1], f32)
            nc.vector.tensor_scalar_mul(negm, amax, -1.0)
            asum = sg.tile([1, 1], f32)
            nc.scalar.activation(arow, arow, AF.Exp, bias=negm, scale=1.0,
                                 accum_out=asum)
            rcp = sg.tile([1, 1], f32)
            nc.vector.reciprocal(rcp, asum)
            nc.vector.tensor_scalar_mul(arow, arow, rcp)
            nc.sync.dma_start(out=attn_o[:, :], in_=arow)
            nc.sync.dma_start(out=areg.ap(), in_=arow)

            # ---------- attn_applied cols: ctx[128i+p] = sum_j aw[j] encT[128i+p, j] ----------
            encT_sb = sg.tile([128, H // 128, L], f32)
            nc.sync.dma_start(
                out=encT_sb, in_=encT.ap().rearrange("(i p) j -> p i j", p=128)
            )
            awb = sg.tile([128, H // 128, L], f32)
            a_ap = areg.ap()
            nc.sync.dma_start(
                out=awb,
                in_=bass.AP(tensor=a_ap.tensor, offset=a_ap.offset,
                            ap=[[0, 128], [0, H // 128], [1, L]]),
            )
            ctxp = sg.tile([128, H // 128, L], f32)
            nc.vector.tensor_mul(ctxp, encT_sb, awb)
            ctxcol = sg.tile([128, H // 128], f32)
            nc.vector.reduce_sum(ctxcol, ctxp, axis=X)
            nc.sync.dma_start(
                out=creg.ap().rearrange("(i p) -> p i", p=128), in_=ctxcol
            )
            crb = sg.tile([128, H], f32)     # attn_applied row, all partitions
            nc.sync.dma_start(out=crb, in_=bc(creg, 128, H))

            # ---------- comb matvec -> x_k (output-sharded) ----------
            cb_sb = sg.tile([128, 2], f32)
            nc.sync.dma_start(out=cb_sb, in_=cb.ap().rearrange("(m p) -> p m", p=128))
            xq = sg.tile([128, 4], f32)      # col hh*2+m: quarter dot products
            for hh in range(2):
                for m in range(2):
                    w_q = gwp.tile([128, H], f32, name=f"wq{hh}{m}", tag="gw")
                    nc.sync.dma_start(
                        out=w_q, in_=cw[m * 128 : (m + 1) * 128, hh * H : (hh + 1) * H]
                    )
                    s_t = scrp.tile([128, H], f32, name=f"cscr{hh}{m}", tag="scr")
                    nc.vector.tensor_mul(s_t, w_q, eb if hh == 0 else crb)
                    nc.scalar.activation(s_t, s_t, AF.Identity,
                                         accum_out=xq[:, hh * 2 + m : hh * 2 + m + 1])
            xpre = sg.tile([128, 2], f32)
            nc.vector.tensor_add(xpre, xq[:, 0:2], xq[:, 2:4])
            x_sb = sg.tile([128, 2], f32)
            for m in range(2):
                nc.scalar.activation(
                    x_sb[:, m : m + 1], xpre[:, m : m + 1], AF.Relu,
                    bias=cb_sb[:, m : m + 1], scale=1.0,
                )
            nc.sync.dma_start(
                out=cc_in1.ap().rearrange("(m p) -> p m", p=128), in_=x_sb
            )

            # ---------- AllGather x ----------
            nc.gpsimd.collective_compute(
                "AllGather", OP.bypass, replica_groups=RG,
                ins=[cc_in1.ap()], outs=[cc_out1.ap()],
            )
            xb = sg.tile([128, H], f32)
            nc.sync.dma_start(out=xb, in_=bc(cc_out1, 128, H))

            # ---------- GRU gates ----------
            bih_sb = sg.tile([128, 6], f32)
            bhh_sb = sg.tile([128, 6], f32)
            for g, nm in enumerate(("bihr", "bihz", "bihn")):
                nc.sync.dma_start(
                    out=bih_sb[:, 2 * g : 2 * g + 2],
                    in_=bg[nm].ap().rearrange("(m p) -> p m", p=128),
                )
            for g, nm in enumerate(("bhhr", "bhhz", "bhhn")):
                nc.sync.dma_start(
                    out=bhh_sb[:, 2 * g : 2 * g + 2],
                    in_=bg[nm].ap().rearrange("(m p) -> p m", p=128),
                )
            gi_sb = sg.tile([128, 6], f32)
            gh_sb = sg.tile([128, 6], f32)
            # gh first: depends only on h0 (overlaps the AllGather wait)
            for g, nm in enumerate(("whhr", "whhz", "whhn")):
                for hhalf in range(2):
                    j = 2 * g + hhalf
                    w_t = gwp.tile([128, H], f32, name=f"wt_hh{j}", tag="gw")
                    nc.sync.dma_start(
                        out=w_t, in_=wg[nm][hhalf * 128 : (hhalf + 1) * 128, :]
                    )
                    s_t = scrp.tile([128, H], f32, name=f"shh{j}", tag="scr")
                    nc.vector.tensor_mul(s_t, w_t, h0b)
                    nc.scalar.activation(s_t, s_t, AF.Identity,
                                         accum_out=gh_sb[:, j : j + 1])
            for g, nm in enumerate(("wihr", "wihz", "wihn")):
                for hhalf in range(2):
                    j = 2 * g + hhalf
                    w_t = gwp.tile([128, H], f32, name=f"wt_ih{j}", tag="gw")
                    nc.sync.dma_start(
                        out=w_t, in_=wg[nm][hhalf * 128 : (hhalf + 1) * 128, :]
                    )
                    s_t = scrp.tile([128, H], f32, name=f"sih{j}", tag="scr")
                    nc.vector.tensor_mul(s_t, w_t, xb)
                    nc.scalar.activation(s_t, s_t, AF.Identity,
                                         accum_out=gi_sb[:, j : j + 1])
            nc.vector.tensor_add(gi_sb, gi_sb, bih_sb)
            nc.vector.tensor_add(gh_sb, gh_sb, bhh_sb)

            # gates: r=sig(gi_r+gh_r) z=sig(gi_z+gh_z) n=tanh(gi_n + r*gh_n)
            rzt = sg.tile([128, 4], f32)
            nc.vector.tensor_add(rzt, gi_sb[:, 0:4], gh_sb[:, 0:4])
            rz = sg.tile([128, 4], f32)
            nc.scalar.activation(rz, rzt, AF.Sigmoid)
            nt = sg.tile([128, 2], f32)
            nc.vector.tensor_mul(nt, rz[:, 0:2], gh_sb[:, 4:6])
            nc.vector.tensor_add(nt, nt, gi_sb[:, 4:6])
            nn_sb = sg.tile([128, 2], f32)
            nc.scalar.activation(nn_sb, nt, AF.Tanh)
            h0k_sb = sg.tile([128, 2], f32)
            nc.sync.dma_start(
                out=h0k_sb, in_=h0k.ap().rearrange("(m p) -> p m", p=128)
            )
            # h' = n + z*(h0 - n)
            d_sb = sg.tile([128, 2], f32)
            nc.vector.tensor_sub(d_sb, h0k_sb, nn_sb)
            nc.vector.tensor_mul(d_sb, rz[:, 2:4], d_sb)
            hn_sb = sg.tile([128, 2], f32)
            nc.vector.tensor_add(hn_sb, nn_sb, d_sb)
            nc.sync.dma_start(
                out=cc_in2.ap().rearrange("(m p) -> p m", p=128), in_=hn_sb
            )

            # ---------- AllGather h_new ----------
            nc.gpsimd.collective_compute(
                "AllGather", OP.bypass, replica_groups=RG,
                ins=[cc_in2.ap()], outs=[cc_out2.ap()],
            )
            nc.sync.dma_start(
                out=hnew_o[:, :], in_=cc_out2.ap().rearrange("(a f) -> a f", a=1)
            )
            hb = sg.tile([128, H], f32)
            nc.sync.dma_start(out=hb, in_=bc(cc_out2, 128, H))

            # ---------- logits (vocab-sharded), split across DVE/GpSimd/ACT ----------
            ob_sb = sg.tile([128, VT], f32)
            nc.sync.dma_start(
                out=ob_sb, in_=ob.ap().rearrange("(t p) -> p t", p=128)
            )
            logit_sb = sg.tile([128, VT], f32)
            nc.vector.memset(logit_sb, 0.0)
            for tt in range(VT // 2):
                w2 = lwp.tile([128, 2, H], f32, name=f"lw{tt}", tag="lw")
                r2 = min(256, VS - tt * 256)
                if r2 == 256:
                    nc.sync.dma_start(
                        out=w2,
                        in_=ow[tt * 256 : (tt + 1) * 256, :].rearrange(
                            "(g p) f -> p g f", p=128
                        ),
                    )
                else:
                    nc.sync.dma_start(
                        out=w2[:, 0, :], in_=ow[tt * 256 : tt * 256 + 128, :]
                    )
                    nc.sync.dma_start(
                        out=w2[: r2 - 128, 1, :], in_=ow[tt * 256 + 128 : VS, :]
                    )
                for g in range(2):
                    t = 2 * tt + g
                    rows = min(128, VS - t * 128)
                    s_t = scrp.tile([128, H], f32, name=f"ls{t}", tag="scr")
                    mul_eng = nc.gpsimd if t % 3 == 2 else nc.vector
                    mul_eng.tensor_mul(s_t[:rows, :], w2[:rows, g, :], hb[:rows, :])
                    if t % 10 == 1:
                        nc.vector.reduce_sum(
                            logit_sb[:rows, t : t + 1], s_t[:rows, :], axis=X
                        )
                    else:
                        nc.scalar.activation(
                            s_t[:rows, :], s_t[:rows, :], AF.Identity,
                            accum_out=logit_sb[:rows, t : t + 1],
                        )
            nc.vector.tensor_add(logit_sb, logit_sb, ob_sb)
            nc.sync.dma_start(out=logits_o[:, :], in_=logit_sb)

    nc.compile()
    return nc


def _marshal(input_ids, hidden, encoder_outputs, emb, attn_w, attn_b,
             comb_w, comb_b, w_ih, w_hh, b_ih, b_hh, out_w, out_b):
    """Host-side sharding: returns one input map per core."""
    f = np.float32
    ii = int(np.asarray(input_ids).ravel()[0])
    erow = np.ascontiguousarray(np.asarray(emb)[ii], dtype=f).reshape(1, H)
    h0 = np.ascontiguousarray(np.asarray(hidden, f).reshape(H))
    awf = np.asarray(attn_w, f)
    ab = np.asarray(attn_b, f).reshape(1, L)
    encT = np.ascontiguousarray(np.asarray(encoder_outputs, f).T)
    cwf = np.asarray(comb_w, f)
    cbf = np.asarray(comb_b, f)
    wihf = np.asarray(w_ih, f)
    whhf = np.asarray(w_hh, f)
    bihf = np.asarray(b_ih, f)
    bhhf = np.asarray(b_hh, f)
    owf = np.asarray(out_w, f)
    obf = np.asarray(out_b, f)

    common = {
        "erow": erow, "aw": awf, "ab": ab, "encT": encT,
        "h0row": h0.reshape(1, H),
    }
    in_maps = []
    for k in range(NC):
        r0 = HS * k
        v0 = VS * k if k < NC - 1 else V - VS
        obk = np.zeros(VPAD, f)
        obk[:VS] = obf[v0 : v0 + VS]
        m = dict(common)
        m["cw"] = cwf[r0 : r0 + HS]
        m["cb"] = cbf[r0 : r0 + HS]
        m["wihr"] = wihf[r0 : r0 + HS]
        m["wihz"] = wihf[H + r0 : H + r0 + HS]
        m["wihn"] = wihf[2 * H + r0 : 2 * H + r0 + HS]
        m["whhr"] = whhf[r0 : r0 + HS]
        m["whhz"] = whhf[H + r0 : H + r0 + HS]
        m["whhn"] = whhf[2 * H + r0 : 2 * H + r0 + HS]
        m["bihr"] = bihf[r0 : r0 + HS]
        m["bihz"] = bihf[H + r0 : H + r0 + HS]
        m["bihn"] = bihf[2 * H + r0 : 2 * H + r0 + HS]
        m["bhhr"] = bhhf[r0 : r0 + HS]
        m["bhhz"] = bhhf[H + r0 : H + r0 + HS]
        m["bhhn"] = bhhf[2 * H + r0 : 2 * H + r0 + HS]
        m["h0k"] = h0[r0 : r0 + HS]
        m["ow"] = owf[v0 : v0 + VS]
        m["ob"] = obk
        in_maps.append(m)
    return in_maps


def kernel(**inputs):
    global LAST_RESULT
    from concourse.bass_utils import run_bass_kernel_spmd

    if "nc" not in _NC_CACHE:
        _NC_CACHE["nc"] = _build()
    nc = _NC_CACHE["nc"]

    in_maps = _marshal(**inputs)

    kwargs = {}
    if TRACE:
        import concourse.bass_utils as bu
        bu.upload_artifacts = lambda d: str(d)
        kwargs = dict(trace=True, trace_cores=[0])
    res = run_bass_kernel_spmd(nc, in_maps, core_ids=list(range(NC)), **kwargs)
    LAST_RESULT = res

    logits = np.empty((1, V), np.float32)
    for k in range(NC):
        v0 = VS * k if k < NC - 1 else V - VS
        arr = res.results[k]["logits"]          # [128, VT]
        logits[0, v0 : v0 + VS] = arr.T.reshape(-1)[:VS]
    hnew = res.results[0]["hnew"].reshape(1, 1, H).astype(np.float32)
    attn = res.results[0]["attn"].reshape(1, L).astype(np.float32)
    return logits, hnew, attn


# revision 5
# speedup vs baseline: 1.1877x; 1.1877x over previous
"""AttnDecoderRNN single-step kernel for 8 Trainium2 NeuronCores.

Sharding (tensor-parallel, per spec hint):
  - attention (tiny) replicated on every core
  - comb_w output-sharded (256 rows/core)  -> AllGather x (1KB/core)
  - w_ih/w_hh output-sharded (768 rows/core, r/z/n aligned) -> AllGather h_new
  - out_w vocab-row-sharded (6283 rows/core, core 7 overlaps core 6)
  - emb: only the one indexed row is needed; gathered on host (8KB)

All matvecs run as DVE/GpSimd elementwise multiply (weight rows on
partitions, activation row broadcast along the free dim) + a reduction
pass on the Scalar engine (activation Identity with accum_out) or DVE
reduce_sum — no weight transposes, no TensorE (whose cold-clock tiny
matmuls measured ~10x slower than the same work on DVE). The kernel is
HBM-bandwidth-bound on the out_w stream.
"""

import numpy as np

H = 2048
V = 50257
L = 20
NC = 8
HS = H // NC          # 256 hidden rows per core
VS = 6283             # out_w rows per core (core 7 re-computes 7 rows)
VT = (VS + 127) // 128  # 50 vocab tiles per core
VPAD = VT * 128       # 6400

TRACE = False
LAST_RESULT = None

# creg holds attn_applied in partition-major order: creg[p*16+i] = ctx[128*i+p].
# comb_w's attn-half columns are permuted on the host to match.
_CREG_PERM = (np.arange(H) % 16) * 128 + np.arange(H) // 16

_NC_CACHE = {}


def _build():
    import concourse.bass as bass
    import concourse.bacc as bacc
    import concourse.tile as tile
    from concourse import mybir

    f32 = mybir.dt.float32
    AF = mybir.ActivationFunctionType
    OP = mybir.AluOpType
    RG = [list(range(NC))]
    X = mybir.AxisListType.X

    nc = bacc.Bacc("TRN2", target_bir_lowering=False, debug=False, num_devices=NC)

    # ---- I/O ----
    erow = nc.dram_tensor("erow", [1, H], f32, kind="ExternalInput")
    aw = nc.dram_tensor("aw", [L, 2 * H], f32, kind="ExternalInput")
    ab = nc.dram_tensor("ab", [1, L], f32, kind="ExternalInput")
    encT = nc.dram_tensor("encT", [H, L], f32, kind="ExternalInput")
    cw = nc.dram_tensor("cw", [HS, 2 * H], f32, kind="ExternalInput")
    cb = nc.dram_tensor("cb", [HS], f32, kind="ExternalInput")
    wg = {
        nm: nc.dram_tensor(nm, [HS, H], f32, kind="ExternalInput")
        for nm in ("wihr", "wihz", "wihn", "whhr", "whhz", "whhn")
    }
    bg = {
        nm: nc.dram_tensor(nm, [HS], f32, kind="ExternalInput")
        for nm in ("bihr", "bihz", "bihn", "bhhr", "bhhz", "bhhn")
    }
    h0row = nc.dram_tensor("h0row", [1, H], f32, kind="ExternalInput")
    h0k = nc.dram_tensor("h0k", [HS], f32, kind="ExternalInput")
    ow = nc.dram_tensor("ow", [VS, H], f32, kind="ExternalInput")
    ob = nc.dram_tensor("ob", [VPAD], f32, kind="ExternalInput")

    logits_o = nc.dram_tensor("logits", [128, VT], f32, kind="ExternalOutput")
    hnew_o = nc.dram_tensor("hnew", [1, H], f32, kind="ExternalOutput")
    attn_o = nc.dram_tensor("attn", [1, L], f32, kind="ExternalOutput")

    # internal DRAM: collective bounces + broadcast staging
    cc_in1 = nc.dram_tensor("cc_in1", [HS], f32)
    cc_out1 = nc.dram_tensor("cc_out1", [H], f32)
    cc_in2 = nc.dram_tensor("cc_in2", [HS], f32)
    cc_out2 = nc.dram_tensor("cc_out2", [H], f32)
    areg = nc.dram_tensor("areg", [L], f32)
    creg = nc.dram_tensor("creg", [H], f32)
    cc_win = nc.dram_tensor("cc_win", [1], f32)
    cc_wout = nc.dram_tensor("cc_wout", [NC], f32)

    def bc(handle, n, cnt):
        # broadcast a contiguous DRAM range across n partitions
        a = handle.ap()
        return bass.AP(tensor=a.tensor, offset=a.offset, ap=[[0, n], [1, cnt]])

    with tile.TileContext(nc) as tc:
        with (
            tc.tile_pool(name="singles", bufs=1) as sg,
            tc.tile_pool(name="gw", bufs=4) as gwp,
            tc.tile_pool(name="lw", bufs=5) as lwp,
            tc.tile_pool(name="scr", bufs=3) as scrp,
        ):
            # warm-up collective: absorbs the CC-stream barrier and the
            # cross-core NEFF start skew before the real AllGathers
            nc.sync.dma_start(out=cc_win.ap(), in_=ab[0:1, 0:1].rearrange("a b -> (a b)"))
            nc.gpsimd.collective_compute(
                "AllGather", OP.bypass, replica_groups=RG,
                ins=[cc_win.ap()], outs=[cc_wout.ap()],
            )

            # broadcast rows used by several phases
            eb = sg.tile([128, H], f32)        # embedded row, all partitions
            nc.sync.dma_start(out=eb, in_=bc(erow, 128, H))
            h0b = sg.tile([128, H], f32)       # h0 row, all partitions
            nc.sync.dma_start(out=h0b, in_=bc(h0row, 128, H))

            # ---------- attention scores: aw @ [emb; h0] ----------
            # rows of attn_w on partitions 0..19, contraction along free dim
            adots = sg.tile([L, 2], f32)
            for hh in range(2):
                awh = gwp.tile([L, H], f32, name=f"awh{hh}", tag="gw")
                nc.sync.dma_start(out=awh, in_=aw[:, hh * H : (hh + 1) * H])
                s_t = scrp.tile([L, H], f32, name=f"ascr{hh}", tag="scr")
                nc.vector.tensor_mul(s_t, awh, (eb if hh == 0 else h0b)[:L, :])
                nc.scalar.activation(s_t, s_t, AF.Identity,
                                     accum_out=adots[:, hh : hh + 1])
            acol = sg.tile([L, 1], f32)
            nc.vector.tensor_add(acol, adots[:, 0:1], adots[:, 1:2])
            # to one row + bias + softmax
            arow = sg.tile([1, L], f32)
            nc.sync.dma_start(out=arow, in_=acol)
            ab_sb = sg.tile([1, L], f32)
            nc.sync.dma_start(out=ab_sb, in_=ab[:, :])
            nc.vector.tensor_add(arow, arow, ab_sb)
            amax = sg.tile([1, 1], f32)
            nc.vector.reduce_max(amax, arow, axis=X)
            negm = sg.tile([1, 1], f32)
            nc.vector.tensor_scalar_mul(negm, amax, -1.0)
            asum = sg.tile([1, 1], f32)
            nc.scalar.activation(arow, arow, AF.Exp, bias=negm, scale=1.0,
                                 accum_out=asum)
            rcp = sg.tile([1, 1], f32)
            nc.vector.reciprocal(rcp, asum)
            nc.vector.tensor_scalar_mul(arow, arow, rcp)
            nc.sync.dma_start(out=attn_o[:, :], in_=arow)
            nc.sync.dma_start(out=areg.ap(), in_=arow)

            # ---------- attn_applied cols: ctx[128i+p] = sum_j aw[j] encT[128i+p, j] ----------
            encT_sb = sg.tile([128, H // 128, L], f32)
            nc.sync.dma_start(
                out=encT_sb, in_=encT.ap().rearrange("(i p) j -> p i j", p=128)
            )
            awb = sg.tile([128, L], f32)
            nc.sync.dma_start(out=awb, in_=bc(areg, 128, L))
            awb_ap = awb[:, :]
            awb3 = bass.AP(tensor=awb_ap.tensor, offset=awb_ap.offset,
                           ap=[awb_ap.ap[0], [0, H // 128], awb_ap.ap[1]])
            ctxp = sg.tile([128, H // 128, L], f32)
            nc.vector.tensor_mul(ctxp, encT_sb, awb3)
            ctxcol = sg.tile([128, H // 128], f32)
            nc.vector.reduce_sum(ctxcol, ctxp, axis=X)
            nc.sync.dma_start(
                out=creg.ap().rearrange("(p i) -> p i", p=128), in_=ctxcol
            )
            crb = sg.tile([128, H], f32)     # attn_applied row, all partitions
            nc.sync.dma_start(out=crb, in_=bc(creg, 128, H))

            # ---------- comb matvec -> x_k (output-sharded) ----------
            cb_sb = sg.tile([128, 2], f32)
            nc.sync.dma_start(out=cb_sb, in_=cb.ap().rearrange("(m p) -> p m", p=128))
            xq = sg.tile([128, 4], f32)      # col hh*2+m: quarter dot products
            for hh in range(2):
                for m in range(2):
                    w_q = gwp.tile([128, H], f32, name=f"wq{hh}{m}", tag="gw")
                    nc.sync.dma_start(
                        out=w_q, in_=cw[m * 128 : (m + 1) * 128, hh * H : (hh + 1) * H]
                    )
                    s_t = scrp.tile([128, H], f32, name=f"cscr{hh}{m}", tag="scr")
                    nc.vector.tensor_mul(s_t, w_q, eb if hh == 0 else crb)
                    nc.scalar.activation(s_t, s_t, AF.Identity,
                                         accum_out=xq[:, hh * 2 + m : hh * 2 + m + 1])
            xpre = sg.tile([128, 2], f32)
            nc.vector.tensor_add(xpre, xq[:, 0:2], xq[:, 2:4])
            x_sb = sg.tile([128, 2], f32)
            for m in range(2):
                nc.scalar.activation(
                    x_sb[:, m : m + 1], xpre[:, m : m + 1], AF.Relu,
                    bias=cb_sb[:, m : m + 1], scale=1.0,
                )
            nc.sync.dma_start(
                out=cc_in1.ap().rearrange("(m p) -> p m", p=128), in_=x_sb
            )

            # ---------- AllGather x ----------
            nc.gpsimd.collective_compute(
                "AllGather", OP.bypass, replica_groups=RG,
                ins=[cc_in1.ap()], outs=[cc_out1.ap()],
            )
            xb = sg.tile([128, H], f32)
            nc.sync.dma_start(out=xb, in_=bc(cc_out1, 128, H))

            # ---------- GRU gates ----------
            bih_sb = sg.tile([128, 6], f32)
            bhh_sb = sg.tile([128, 6], f32)
            for g, nm in enumerate(("bihr", "bihz", "bihn")):
                nc.sync.dma_start(
                    out=bih_sb[:, 2 * g : 2 * g + 2],
                    in_=bg[nm].ap().rearrange("(m p) -> p m", p=128),
                )
            for g, nm in enumerate(("bhhr", "bhhz", "bhhn")):
                nc.sync.dma_start(
                    out=bhh_sb[:, 2 * g : 2 * g + 2],
                    in_=bg[nm].ap().rearrange("(m p) -> p m", p=128),
                )
            gi_sb = sg.tile([128, 6], f32)
            gh_sb = sg.tile([128, 6], f32)
            # gh first: depends only on h0 (overlaps the AllGather wait)
            for g, nm in enumerate(("whhr", "whhz", "whhn")):
                for hhalf in range(2):
                    j = 2 * g + hhalf
                    w_t = gwp.tile([128, H], f32, name=f"wt_hh{j}", tag="gw")
                    nc.sync.dma_start(
                        out=w_t, in_=wg[nm][hhalf * 128 : (hhalf + 1) * 128, :]
                    )
                    s_t = scrp.tile([128, H], f32, name=f"shh{j}", tag="scr")
                    nc.vector.tensor_mul(s_t, w_t, h0b)
                    nc.scalar.activation(s_t, s_t, AF.Identity,
                                         accum_out=gh_sb[:, j : j + 1])
            for g, nm in enumerate(("wihr", "wihz", "wihn")):
                for hhalf in range(2):
                    j = 2 * g + hhalf
                    w_t = gwp.tile([128, H], f32, name=f"wt_ih{j}", tag="gw")
                    nc.sync.dma_start(
                        out=w_t, in_=wg[nm][hhalf * 128 : (hhalf + 1) * 128, :]
                    )
                    s_t = scrp.tile([128, H], f32, name=f"sih{j}", tag="scr")
                    nc.vector.tensor_mul(s_t, w_t, xb)
                    nc.scalar.activation(s_t, s_t, AF.Identity,
                                         accum_out=gi_sb[:, j : j + 1])
            nc.vector.tensor_add(gi_sb, gi_sb, bih_sb)
            nc.vector.tensor_add(gh_sb, gh_sb, bhh_sb)

            # gates: r=sig(gi_r+gh_r) z=sig(gi_z+gh_z) n=tanh(gi_n + r*gh_n)
            rzt = sg.tile([128, 4], f32)
            nc.vector.tensor_add(rzt, gi_sb[:, 0:4], gh_sb[:, 0:4])
            rz = sg.tile([128, 4], f32)
            nc.scalar.activation(rz, rzt, AF.Sigmoid)
            nt = sg.tile([128, 2], f32)
            nc.vector.tensor_mul(nt, rz[:, 0:2], gh_sb[:, 4:6])
            nc.vector.tensor_add(nt, nt, gi_sb[:, 4:6])
            nn_sb = sg.tile([128, 2], f32)
            nc.scalar.activation(nn_sb, nt, AF.Tanh)
            h0k_sb = sg.tile([128, 2], f32)
            nc.sync.dma_start(
                out=h0k_sb, in_=h0k.ap().rearrange("(m p) -> p m", p=128)
            )
            # h' = n + z*(h0 - n)
            d_sb = sg.tile([128, 2], f32)
            nc.vector.tensor_sub(d_sb, h0k_sb, nn_sb)
            nc.vector.tensor_mul(d_sb, rz[:, 2:4], d_sb)
            hn_sb = sg.tile([128, 2], f32)
            nc.vector.tensor_add(hn_sb, nn_sb, d_sb)
            nc.sync.dma_start(
                out=cc_in2.ap().rearrange("(m p) -> p m", p=128), in_=hn_sb
            )

            # ---------- AllGather h_new ----------
            nc.gpsimd.collective_compute(
                "AllGather", OP.bypass, replica_groups=RG,
                ins=[cc_in2.ap()], outs=[cc_out2.ap()],
            )
            nc.sync.dma_start(
                out=hnew_o[:, :], in_=cc_out2.ap().rearrange("(a f) -> a f", a=1)
            )
            hb = sg.tile([128, H], f32)
            nc.sync.dma_start(out=hb, in_=bc(cc_out2, 128, H))

            # ---------- logits (vocab-sharded), split across DVE/GpSimd/ACT ----------
            ob_sb = sg.tile([128, VT], f32)
            nc.sync.dma_start(
                out=ob_sb, in_=ob.ap().rearrange("(t p) -> p t", p=128)
            )
            logit_sb = sg.tile([128, VT], f32)
            nc.vector.memset(logit_sb, 0.0)
            for tt in range(VT // 2):
                w2 = lwp.tile([128, 2, H], f32, name=f"lw{tt}", tag="lw")
                r2 = min(256, VS - tt * 256)
                if r2 == 256:
                    nc.sync.dma_start(
                        out=w2,
                        in_=ow[tt * 256 : (tt + 1) * 256, :].rearrange(
                            "(g p) f -> p g f", p=128
                        ),
                    )
                else:
                    nc.sync.dma_start(
                        out=w2[:, 0, :], in_=ow[tt * 256 : tt * 256 + 128, :]
                    )
                    nc.sync.dma_start(
                        out=w2[: r2 - 128, 1, :], in_=ow[tt * 256 + 128 : VS, :]
                    )
                for g in range(2):
                    t = 2 * tt + g
                    rows = min(128, VS - t * 128)
                    s_t = scrp.tile([128, H], f32, name=f"ls{t}", tag="scr")
                    mul_eng = nc.gpsimd if t % 4 == 3 else nc.vector
                    mul_eng.tensor_mul(s_t[:rows, :], w2[:rows, g, :], hb[:rows, :])
                    if t in (1, 21, 41):
                        nc.vector.reduce_sum(
                            logit_sb[:rows, t : t + 1], s_t[:rows, :], axis=X
                        )
                    else:
                        nc.scalar.activation(
                            s_t[:rows, :], s_t[:rows, :], AF.Identity,
                            accum_out=logit_sb[:rows, t : t + 1],
                        )
            nc.vector.tensor_add(logit_sb, logit_sb, ob_sb)
            nc.sync.dma_start(out=logits_o[:, :], in_=logit_sb)

    nc.compile()
    return nc


def _marshal(input_ids, hidden, encoder_outputs, emb, attn_w, attn_b,
             comb_w, comb_b, w_ih, w_hh, b_ih, b_hh, out_w, out_b):
    """Host-side sharding: returns one input map per core."""
    f = np.float32
    ii = int(np.asarray(input_ids).ravel()[0])
    erow = np.ascontiguousarray(np.asarray(emb)[ii], dtype=f).reshape(1, H)
    h0 = np.ascontiguousarray(np.asarray(hidden, f).reshape(H))
    awf = np.asarray(attn_w, f)
    ab = np.asarray(attn_b, f).reshape(1, L)
    encT = np.ascontiguousarray(np.asarray(encoder_outputs, f).T)
    cwf = np.asarray(comb_w, f)
    cbf = np.asarray(comb_b, f)
    wihf = np.asarray(w_ih, f)
    whhf = np.asarray(w_hh, f)
    bihf = np.asarray(b_ih, f)
    bhhf = np.asarray(b_hh, f)
    owf = np.asarray(out_w, f)
    obf = np.asarray(out_b, f)

    common = {
        "erow": erow, "aw": awf, "ab": ab, "encT": encT,
        "h0row": h0.reshape(1, H),
    }
    in_maps = []
    for k in range(NC):
        r0 = HS * k
        v0 = VS * k if k < NC - 1 else V - VS
        obk = np.zeros(VPAD, f)
        obk[:VS] = obf[v0 : v0 + VS]
        m = dict(common)
        m["cw"] = np.concatenate(
            [cwf[r0 : r0 + HS, :H], cwf[r0 : r0 + HS, H:][:, _CREG_PERM]], axis=1
        )
        m["cb"] = cbf[r0 : r0 + HS]
        m["wihr"] = wihf[r0 : r0 + HS]
        m["wihz"] = wihf[H + r0 : H + r0 + HS]
        m["wihn"] = wihf[2 * H + r0 : 2 * H + r0 + HS]
        m["whhr"] = whhf[r0 : r0 + HS]
        m["whhz"] = whhf[H + r0 : H + r0 + HS]
        m["whhn"] = whhf[2 * H + r0 : 2 * H + r0 + HS]
        m["bihr"] = bihf[r0 : r0 + HS]
        m["bihz"] = bihf[H + r0 : H + r0 + HS]
        m["bihn"] = bihf[2 * H + r0 : 2 * H + r0 + HS]
        m["bhhr"] = bhhf[r0 : r0 + HS]
        m["bhhz"] = bhhf[H + r0 : H + r0 + HS]
        m["bhhn"] = bhhf[2 * H + r0 : 2 * H + r0 + HS]
        m["h0k"] = h0[r0 : r0 + HS]
        m["ow"] = owf[v0 : v0 + VS]
        m["ob"] = obk
        in_maps.append(m)
    return in_maps


def kernel(**inputs):
    global LAST_RESULT
    from concourse.bass_utils import run_bass_kernel_spmd

    if "nc" not in _NC_CACHE:
        _NC_CACHE["nc"] = _build()
    nc = _NC_CACHE["nc"]

    in_maps = _marshal(**inputs)

    kwargs = {}
    if TRACE:
        import concourse.bass_utils as bu
        bu.upload_artifacts = lambda d: str(d)
        kwargs = dict(trace=True, trace_cores=[0])
    res = run_bass_kernel_spmd(nc, in_maps, core_ids=list(range(NC)), **kwargs)
    LAST_RESULT = res

    logits = np.empty((1, V), np.float32)
    for k in range(NC):
        v0 = VS * k if k < NC - 1 else V - VS
        arr = res.results[k]["logits"]          # [128, VT]
        logits[0, v0 : v0 + VS] = arr.T.reshape(-1)[:VS]
    hnew = res.results[0]["hnew"].reshape(1, 1, H).astype(np.float32)
    attn = res.results[0]["attn"].reshape(1, L).astype(np.float32)
    return logits, hnew, attn


# revision 8
# speedup vs baseline: 1.3031x; 1.0971x over previous
"""AttnDecoderRNN single-step kernel for 8 Trainium2 NeuronCores.

Two-phase zero-collective design. Measured traces showed every on-device
collective pays a CC-stream rendezvous barrier dominated by cross-core
NEFF start skew (~60-100us), so the h_new mixing is done through a tiny
host round-trip instead of AllGathers:

  K1 (per core k, no cross-core deps):
    - attention (replicated, tiny)
    - x_k = relu(comb_w[k-rows] @ [emb; ctx])           (output-sharded)
    - partial gate sums: gi_part = w_ih[:, k-cols] @ x_k,
      gh_part = w_hh[:, k-cols] @ h0_k                  (contraction-sharded)
    -> outputs arg[12288] = [gi_part; gh_part] (48KB)
  host: argsum = sum_k arg_k  (the "AllReduce", 100K adds)
  K2 (per core k):
    - gates + h_new from argsum (replicated, tiny)
    - logits_k = h_new @ out_w[k-rows].T + out_b[k-rows] (vocab-sharded)
  host: concat logits slices.

All matvecs run as DVE/GpSimd elementwise multiply (weight rows on
partitions, activation row broadcast along the free dim) + a reduction
on the Scalar engine (activation Identity accum_out) or DVE reduce_sum.
No TensorE: its cold-clock tiny matmuls measured ~10x slower than the
same work on DVE. The kernel is HBM-bandwidth-bound on the out_w stream.
"""

import numpy as np

H = 2048
V = 50257
L = 20
NC = 8
HS = H // NC          # 256 hidden cols per core (contraction shard)
GR = 3 * H            # 6144 gate rows
VS = 6283             # out_w rows per core (core 7 re-computes 7 rows)
VT = (VS + 127) // 128  # 50 vocab tiles per core
VPAD = VT * 128       # 6400

TRACE = False
LAST_RESULT = None

# creg holds attn_applied in partition-major order: creg[p*16+i] = ctx[128*i+p].
# comb_w's attn-half columns are permuted on the host to match.
_CREG_PERM = (np.arange(H) % 16) * 128 + np.arange(H) // 16
# xreg is stored partition-major: xreg[2p+m] = x_k[128m+p]; w_ih's column
# slice is permuted on the host to match.
_XREG_PERM = (np.arange(HS) % 2) * 128 + np.arange(HS) // 2

_NC_CACHE = {}


def _build_k1():
    import concourse.bass as bass
    import concourse.bacc as bacc
    import concourse.tile as tile
    from concourse import mybir

    f32 = mybir.dt.float32
    AF = mybir.ActivationFunctionType
    X = mybir.AxisListType.X

    nc = bacc.Bacc("TRN2", target_bir_lowering=False, debug=False, num_devices=NC)

    erow = nc.dram_tensor("erow", [1, H], f32, kind="ExternalInput")
    aw = nc.dram_tensor("aw", [L, 2 * H], f32, kind="ExternalInput")
    ab = nc.dram_tensor("ab", [1, L], f32, kind="ExternalInput")
    encT = nc.dram_tensor("encT", [H, L], f32, kind="ExternalInput")
    cw = nc.dram_tensor("cw", [HS, 2 * H], f32, kind="ExternalInput")
    cb = nc.dram_tensor("cb", [HS], f32, kind="ExternalInput")
    wihC = nc.dram_tensor("wihC", [GR, HS], f32, kind="ExternalInput")
    whhC = nc.dram_tensor("whhC", [GR, HS], f32, kind="ExternalInput")
    h0row = nc.dram_tensor("h0row", [1, H], f32, kind="ExternalInput")
    h0k = nc.dram_tensor("h0k", [HS], f32, kind="ExternalInput")

    arg_o = nc.dram_tensor("arg", [2 * GR], f32, kind="ExternalOutput")
    attn_o = nc.dram_tensor("attn", [1, L], f32, kind="ExternalOutput")

    areg = nc.dram_tensor("areg", [L], f32)
    creg = nc.dram_tensor("creg", [H], f32)
    xreg = nc.dram_tensor("xreg", [HS], f32)

    def bc(handle, n, cnt):
        a = handle.ap()
        return bass.AP(tensor=a.tensor, offset=a.offset, ap=[[0, n], [1, cnt]])

    NW = GR // 1024  # 6 wide tiles per gate-weight matrix

    with tile.TileContext(nc) as tc:
        with (
            tc.tile_pool(name="singles", bufs=1) as sg,
            tc.tile_pool(name="gw", bufs=4) as gwp,
            tc.tile_pool(name="scr", bufs=3) as scrp,
        ):
            eb = sg.tile([128, H], f32)
            nc.sync.dma_start(out=eb, in_=bc(erow, 128, H))
            h0b = sg.tile([128, H], f32)
            nc.sync.dma_start(out=h0b, in_=bc(h0row, 128, H))
            # h0 contraction-slice broadcast for the gh partials (ready at t=0)
            hkb = sg.tile([128, HS], f32)
            nc.sync.dma_start(out=hkb, in_=bc(h0k, 128, HS))

            # ---- partial gh: w_hh[:, k-cols] rows on partitions ----
            arg_sb = sg.tile([128, 2 * GR // 128], f32)   # [128, 96] lanes j=...
            for jt in range(NW):
                w_t = gwp.tile([128, 8, HS], f32, name=f"whc{jt}", tag="gw")
                nc.sync.dma_start(
                    out=w_t,
                    in_=whhC[1024 * jt : 1024 * (jt + 1), :].rearrange(
                        "(p g) c -> p g c", p=128
                    ),
                )
                s_t = scrp.tile([128, 8, HS], f32, name=f"shc{jt}", tag="scr")
                hk_ap = hkb[:, :]
                hk3 = bass.AP(tensor=hk_ap.tensor, offset=hk_ap.offset,
                              ap=[hk_ap.ap[0], [0, 8], hk_ap.ap[1]])
                nc.vector.tensor_mul(s_t, w_t, hk3)
                for g in range(8):
                    nc.scalar.activation(
                        s_t[:, g, :], s_t[:, g, :], AF.Identity,
                        accum_out=arg_sb[:, 48 + 8 * jt + g : 48 + 8 * jt + g + 1],
                    )

            # ---------- attention ----------
            adots = sg.tile([L, 2], f32)
            for hh in range(2):
                awh = gwp.tile([L, H], f32, name=f"awh{hh}", tag="gw")
                nc.sync.dma_start(out=awh, in_=aw[:, hh * H : (hh + 1) * H])
                s_t = scrp.tile([L, H], f32, name=f"ascr{hh}", tag="scr")
                nc.vector.tensor_mul(s_t, awh, (eb if hh == 0 else h0b)[:L, :])
                nc.scalar.activation(s_t, s_t, AF.Identity,
                                     accum_out=adots[:, hh : hh + 1])
            acol = sg.tile([L, 1], f32)
            nc.vector.tensor_add(acol, adots[:, 0:1], adots[:, 1:2])
            arow = sg.tile([1, L], f32)
            nc.sync.dma_start(out=arow, in_=acol)
            ab_sb = sg.tile([1, L], f32)
            nc.sync.dma_start(out=ab_sb, in_=ab[:, :])
            nc.vector.tensor_add(arow, arow, ab_sb)
            amax = sg.tile([1, 1], f32)
            nc.vector.reduce_max(amax, arow, axis=X)
            negm = sg.tile([1, 1], f32)
            nc.vector.tensor_scalar_mul(negm, amax, -1.0)
            asum = sg.tile([1, 1], f32)
            nc.scalar.activation(arow, arow, AF.Exp, bias=negm, scale=1.0,
                                 accum_out=asum)
            rcp = sg.tile([1, 1], f32)
            nc.vector.reciprocal(rcp, asum)
            nc.vector.tensor_scalar_mul(arow, arow, rcp)
            nc.sync.dma_start(out=attn_o[:, :], in_=arow)
            nc.sync.dma_start(out=areg.ap(), in_=arow)

            # ---------- attn_applied (partition-major creg layout) ----------
            encT_sb = sg.tile([128, H // 128, L], f32)
            nc.sync.dma_start(
                out=encT_sb, in_=encT.ap().rearrange("(i p) j -> p i j", p=128)
            )
            awb = sg.tile([128, L], f32)
            nc.sync.dma_start(out=awb, in_=bc(areg, 128, L))
            awb_ap = awb[:, :]
            awb3 = bass.AP(tensor=awb_ap.tensor, offset=awb_ap.offset,
                           ap=[awb_ap.ap[0], [0, H // 128], awb_ap.ap[1]])
            ctxp = sg.tile([128, H // 128, L], f32)
            nc.vector.tensor_mul(ctxp, encT_sb, awb3)
            ctxcol = sg.tile([128, H // 128], f32)
            nc.vector.reduce_sum(ctxcol, ctxp, axis=X)
            nc.sync.dma_start(
                out=creg.ap().rearrange("(p i) -> p i", p=128), in_=ctxcol
            )
            crb = sg.tile([128, H], f32)
            nc.sync.dma_start(out=crb, in_=bc(creg, 128, H))

            # ---------- comb matvec -> x_k ----------
            cb_sb = sg.tile([128, 2], f32)
            nc.sync.dma_start(out=cb_sb, in_=cb.ap().rearrange("(m p) -> p m", p=128))
            xq = sg.tile([128, 4], f32)
            for hh in range(2):
                for m in range(2):
                    w_q = gwp.tile([128, H], f32, name=f"wq{hh}{m}", tag="gw")
                    nc.sync.dma_start(
                        out=w_q, in_=cw[m * 128 : (m + 1) * 128, hh * H : (hh + 1) * H]
                    )
                    s_t = scrp.tile([128, H], f32, name=f"cscr{hh}{m}", tag="scr")
                    nc.vector.tensor_mul(s_t, w_q, eb if hh == 0 else crb)
                    nc.scalar.activation(s_t, s_t, AF.Identity,
                                         accum_out=xq[:, hh * 2 + m : hh * 2 + m + 1])
            xpre = sg.tile([128, 2], f32)
            nc.vector.tensor_add(xpre, xq[:, 0:2], xq[:, 2:4])
            x_sb = sg.tile([128, 2], f32)
            for m in range(2):
                nc.scalar.activation(
                    x_sb[:, m : m + 1], xpre[:, m : m + 1], AF.Relu,
                    bias=cb_sb[:, m : m + 1], scale=1.0,
                )
            nc.sync.dma_start(
                out=xreg.ap().rearrange("(p m) -> p m", p=128), in_=x_sb
            )
            xkb = sg.tile([128, HS], f32)
            nc.sync.dma_start(out=xkb, in_=bc(xreg, 128, HS))

            # ---- partial gi: w_ih[:, k-cols] ----
            for jt in range(NW):
                w_t = gwp.tile([128, 8, HS], f32, name=f"wic{jt}", tag="gw")
                nc.sync.dma_start(
                    out=w_t,
                    in_=wihC[1024 * jt : 1024 * (jt + 1), :].rearrange(
                        "(p g) c -> p g c", p=128
                    ),
                )
                s_t = scrp.tile([128, 8, HS], f32, name=f"sic{jt}", tag="scr")
                xk_ap = xkb[:, :]
                xk3 = bass.AP(tensor=xk_ap.tensor, offset=xk_ap.offset,
                              ap=[xk_ap.ap[0], [0, 8], xk_ap.ap[1]])
                nc.vector.tensor_mul(s_t, w_t, xk3)
                for g in range(8):
                    nc.scalar.activation(
                        s_t[:, g, :], s_t[:, g, :], AF.Identity,
                        accum_out=arg_sb[:, 8 * jt + g : 8 * jt + g + 1],
                    )

            # store partials partition-major; host reorders to natural j
            nc.sync.dma_start(
                out=arg_o.ap().rearrange("(p col) -> p col", p=128), in_=arg_sb
            )

    nc.compile()
    return nc


def _build_k2():
    import concourse.bass as bass
    import concourse.bacc as bacc
    import concourse.tile as tile
    from concourse import mybir

    f32 = mybir.dt.float32
    AF = mybir.ActivationFunctionType
    X = mybir.AxisListType.X

    nc = bacc.Bacc("TRN2", target_bir_lowering=False, debug=False, num_devices=NC)

    argsum = nc.dram_tensor("argsum", [2 * GR], f32, kind="ExternalInput")
    bih = nc.dram_tensor("bih", [GR], f32, kind="ExternalInput")
    bhh = nc.dram_tensor("bhh", [GR], f32, kind="ExternalInput")
    h0v = nc.dram_tensor("h0v", [H], f32, kind="ExternalInput")
    ow = nc.dram_tensor("ow", [VS, H], f32, kind="ExternalInput")
    ob = nc.dram_tensor("ob", [VPAD], f32, kind="ExternalInput")

    logits_o = nc.dram_tensor("logits", [128, VT], f32, kind="ExternalOutput")
    hnew_o = nc.dram_tensor("hnew", [1, H], f32, kind="ExternalOutput")

    hreg = nc.dram_tensor("hreg", [H], f32)

    def bc(handle, n, cnt):
        a = handle.ap()
        return bass.AP(tensor=a.tensor, offset=a.offset, ap=[[0, n], [1, cnt]])

    def lanes(handle, base):
        # [128, 16] view of handle[base : base+2048], lane (p,c) = elem 16p+c
        a = handle.ap()
        return bass.AP(tensor=a.tensor, offset=a.offset + base,
                       ap=[[16, 128], [1, 16]])

    with tile.TileContext(nc) as tc:
        with (
            tc.tile_pool(name="singles", bufs=1) as sg,
            tc.tile_pool(name="lw", bufs=7) as lwp,
            tc.tile_pool(name="scr", bufs=3) as scrp,
        ):
            # ---------- gates + h_new (lanes j = 16p + c) ----------
            gi = [sg.tile([128, 16], f32, name=f"gi{b}") for b in range(3)]
            gh = [sg.tile([128, 16], f32, name=f"gh{b}") for b in range(3)]
            for b in range(3):
                nc.sync.dma_start(out=gi[b], in_=lanes(argsum, b * H))
                nc.sync.dma_start(out=gh[b], in_=lanes(argsum, GR + b * H))
            bi = [sg.tile([128, 16], f32, name=f"bi{b}") for b in range(3)]
            bh = [sg.tile([128, 16], f32, name=f"bh{b}") for b in range(3)]
            for b in range(3):
                nc.sync.dma_start(out=bi[b], in_=lanes(bih, b * H))
                nc.sync.dma_start(out=bh[b], in_=lanes(bhh, b * H))
            h0t = sg.tile([128, 16], f32)
            nc.sync.dma_start(out=h0t, in_=lanes(h0v, 0))

            for b in range(3):
                nc.vector.tensor_add(gi[b], gi[b], bi[b])
                nc.vector.tensor_add(gh[b], gh[b], bh[b])
            # r, z
            rt = sg.tile([128, 16], f32)
            nc.vector.tensor_add(rt, gi[0], gh[0])
            r_sb = sg.tile([128, 16], f32)
            nc.scalar.activation(r_sb, rt, AF.Sigmoid)
            zt = sg.tile([128, 16], f32)
            nc.vector.tensor_add(zt, gi[1], gh[1])
            z_sb = sg.tile([128, 16], f32)
            nc.scalar.activation(z_sb, zt, AF.Sigmoid)
            # n
            nt = sg.tile([128, 16], f32)
            nc.vector.tensor_mul(nt, r_sb, gh[2])
            nc.vector.tensor_add(nt, nt, gi[2])
            n_sb = sg.tile([128, 16], f32)
            nc.scalar.activation(n_sb, nt, AF.Tanh)
            # h' = n + z*(h0 - n)
            d_sb = sg.tile([128, 16], f32)
            nc.vector.tensor_sub(d_sb, h0t, n_sb)
            nc.vector.tensor_mul(d_sb, z_sb, d_sb)
            hn_sb = sg.tile([128, 16], f32)
            nc.vector.tensor_add(hn_sb, n_sb, d_sb)
            nc.sync.dma_start(
                out=hreg.ap().rearrange("(p c) -> p c", p=128), in_=hn_sb
            )
            nc.sync.dma_start(
                out=hnew_o[:, :], in_=hreg.ap().rearrange("(a f) -> a f", a=1)
            )
            hb = sg.tile([128, H], f32)
            nc.sync.dma_start(out=hb, in_=bc(hreg, 128, H))

            # ---------- logits ----------
            ob_sb = sg.tile([128, VT], f32)
            nc.sync.dma_start(
                out=ob_sb, in_=ob.ap().rearrange("(t p) -> p t", p=128)
            )
            logit_sb = sg.tile([128, VT], f32)
            nc.vector.memset(logit_sb, 0.0)
            for tt in range(VT // 2):
                w2 = lwp.tile([128, 2, H], f32, name=f"lw{tt}", tag="lw")
                r2 = min(256, VS - tt * 256)
                if r2 == 256:
                    nc.sync.dma_start(
                        out=w2,
                        in_=ow[tt * 256 : (tt + 1) * 256, :].rearrange(
                            "(g p) f -> p g f", p=128
                        ),
                    )
                else:
                    nc.sync.dma_start(
                        out=w2[:, 0, :], in_=ow[tt * 256 : tt * 256 + 128, :]
                    )
                    nc.sync.dma_start(
                        out=w2[: r2 - 128, 1, :], in_=ow[tt * 256 + 128 : VS, :]
                    )
                for g in range(2):
                    t = 2 * tt + g
                    rows = min(128, VS - t * 128)
                    s_t = scrp.tile([128, H], f32, name=f"ls{t}", tag="scr")
                    mul_eng = nc.gpsimd if t % 4 == 3 else nc.vector
                    mul_eng.tensor_mul(s_t[:rows, :], w2[:rows, g, :], hb[:rows, :])
                    if t in (1, 21, 41):
                        nc.vector.reduce_sum(
                            logit_sb[:rows, t : t + 1], s_t[:rows, :], axis=X
                        )
                    else:
                        nc.scalar.activation(
                            s_t[:rows, :], s_t[:rows, :], AF.Identity,
                            accum_out=logit_sb[:rows, t : t + 1],
                        )
            nc.vector.tensor_add(logit_sb, logit_sb, ob_sb)
            nc.sync.dma_start(out=logits_o[:, :], in_=logit_sb)

    nc.compile()
    return nc


def _marshal_k1(input_ids, hidden, encoder_outputs, emb, attn_w, attn_b,
                comb_w, comb_b, w_ih, w_hh, out_w=None, out_b=None,
                b_ih=None, b_hh=None):
    f = np.float32
    ii = int(np.asarray(input_ids).ravel()[0])
    erow = np.ascontiguousarray(np.asarray(emb)[ii], dtype=f).reshape(1, H)
    h0 = np.ascontiguousarray(np.asarray(hidden, f).reshape(H))
    common = {
        "erow": erow,
        "aw": np.asarray(attn_w, f),
        "ab": np.asarray(attn_b, f).reshape(1, L),
        "encT": np.ascontiguousarray(np.asarray(encoder_outputs, f).T),
        "h0row": h0.reshape(1, H),
    }
    cwf = np.asarray(comb_w, f)
    cbf = np.asarray(comb_b, f)
    wihf = np.asarray(w_ih, f)
    whhf = np.asarray(w_hh, f)
    in_maps = []
    for k in range(NC):
        r0 = HS * k
        m = dict(common)
        m["cw"] = np.concatenate(
            [cwf[r0 : r0 + HS, :H], cwf[r0 : r0 + HS, H:][:, _CREG_PERM]], axis=1
        )
        m["cb"] = cbf[r0 : r0 + HS]
        m["wihC"] = np.ascontiguousarray(wihf[:, r0 : r0 + HS][:, _XREG_PERM])
        m["whhC"] = np.ascontiguousarray(whhf[:, r0 : r0 + HS])
        m["h0k"] = h0[r0 : r0 + HS]
        in_maps.append(m)
    return in_maps


def _marshal_k2(argsum, hidden, b_ih, b_hh, out_w, out_b):
    f = np.float32
    h0 = np.ascontiguousarray(np.asarray(hidden, f).reshape(H))
    owf = np.asarray(out_w, f)
    obf = np.asarray(out_b, f)
    common = {
        "argsum": argsum,
        "bih": np.asarray(b_ih, f),
        "bhh": np.asarray(b_hh, f),
        "h0v": h0,
    }
    in_maps = []
    for k in range(NC):
        v0 = VS * k if k < NC - 1 else V - VS
        obk = np.zeros(VPAD, f)
        obk[:VS] = obf[v0 : v0 + VS]
        m = dict(common)
        m["ow"] = owf[v0 : v0 + VS]
        m["ob"] = obk
        in_maps.append(m)
    return in_maps


def kernel(**inputs):
    global LAST_RESULT
    from concourse.bass_utils import run_bass_kernel_spmd

    if "k1" not in _NC_CACHE:
        _NC_CACHE["k1"] = _build_k1()
        _NC_CACHE["k2"] = _build_k2()
    nc1, nc2 = _NC_CACHE["k1"], _NC_CACHE["k2"]

    kwargs = {}
    if TRACE:
        import concourse.bass_utils as bu
        bu.upload_artifacts = lambda d: str(d)
        kwargs = dict(trace=True, trace_cores=[0])

    in1 = _marshal_k1(
        inputs["input_ids"], inputs["hidden"], inputs["encoder_outputs"],
        inputs["emb"], inputs["attn_w"], inputs["attn_b"],
        inputs["comb_w"], inputs["comb_b"], inputs["w_ih"], inputs["w_hh"],
    )
    res1 = run_bass_kernel_spmd(nc1, in1, core_ids=list(range(NC)), **kwargs)

    arg_p = np.zeros(2 * GR, np.float32)
    for k in range(NC):
        arg_p += res1.results[k]["arg"].reshape(-1)
    # device layout [p, half, jt, g] -> natural j = jt*1024 + 8p + g per half
    argsum = np.ascontiguousarray(
        arg_p.reshape(128, 2, GR // 1024, 8).transpose(1, 2, 0, 3).reshape(-1)
    )

    in2 = _marshal_k2(argsum, inputs["hidden"], inputs["b_ih"], inputs["b_hh"],
                      inputs["out_w"], inputs["out_b"])
    res2 = run_bass_kernel_spmd(nc2, in2, core_ids=list(range(NC)), **kwargs)
    LAST_RESULT = (res1, res2)

    logits = np.empty((1, V), np.float32)
    for k in range(NC):
        v0 = VS * k if k < NC - 1 else V - VS
        arr = res2.results[k]["logits"]          # [128, VT]
        logits[0, v0 : v0 + VS] = arr.T.reshape(-1)[:VS]
    hnew = res2.results[0]["hnew"].reshape(1, 1, H).astype(np.float32)
    attn = res1.results[0]["attn"].reshape(1, L).astype(np.float32)
    return logits, hnew, attn


# revision 10
# speedup vs baseline: 1.3985x; 1.0732x over previous
"""AttnDecoderRNN single-step kernel for 8 Trainium2 NeuronCores.

Two-phase zero-collective design. Measured traces showed every on-device
collective pays a CC-stream rendezvous barrier dominated by cross-core
NEFF start skew (~60-100us), so the h_new mixing is done through a tiny
host round-trip instead of AllGathers:

  K1 (per core k, no cross-core deps):
    - attention (replicated, tiny)
    - x_k = relu(comb_w[k-rows] @ [emb; ctx])           (output-sharded)
    - partial gate sums: gi_part = w_ih[:, k-cols] @ x_k,
      gh_part = w_hh[:, k-cols] @ h0_k                  (contraction-sharded)
    -> outputs arg[12288] = [gi_part; gh_part] (48KB)
  host: argsum = sum_k arg_k  (the "AllReduce", 100K adds)
  K2 (per core k):
    - gates + h_new from argsum (replicated, tiny)
    - logits_k = h_new @ out_w[k-rows].T + out_b[k-rows] (vocab-sharded)
  host: concat logits slices.

All matvecs run as DVE/GpSimd elementwise multiply (weight rows on
partitions, activation row broadcast along the free dim) + a reduction
on the Scalar engine (activation Identity accum_out) or DVE reduce_sum.
No TensorE: its cold-clock tiny matmuls measured ~10x slower than the
same work on DVE. The kernel is HBM-bandwidth-bound on the out_w stream.
"""

import numpy as np

H = 2048
V = 50257
L = 20
NC = 8
HS = H // NC          # 256 hidden cols per core (contraction shard)
GR = 3 * H            # 6144 gate rows
VS = 6283             # out_w rows per core (core 7 re-computes 7 rows)
VT = (VS + 127) // 128  # 50 vocab tiles per core
VPAD = VT * 128       # 6400

TRACE = False
LAST_RESULT = None

# creg holds attn_applied in partition-major order: creg[p*16+i] = ctx[128*i+p].
# comb_w's attn-half columns are permuted on the host to match.
_CREG_PERM = (np.arange(H) % 16) * 128 + np.arange(H) // 16
# xreg is stored partition-major: xreg[2p+m] = x_k[128m+p]; w_ih's column
# slice is permuted on the host to match.
_XREG_PERM = (np.arange(HS) % 2) * 128 + np.arange(HS) // 2

_NC_CACHE = {}


def _build_k1():
    import concourse.bass as bass
    import concourse.bacc as bacc
    import concourse.tile as tile
    from concourse import mybir

    f32 = mybir.dt.float32
    AF = mybir.ActivationFunctionType
    X = mybir.AxisListType.X

    nc = bacc.Bacc("TRN2", target_bir_lowering=False, debug=False, num_devices=NC)

    erow = nc.dram_tensor("erow", [1, H], f32, kind="ExternalInput")
    aw = nc.dram_tensor("aw", [L, 2 * H], f32, kind="ExternalInput")
    ab = nc.dram_tensor("ab", [1, L], f32, kind="ExternalInput")
    encT = nc.dram_tensor("encT", [H, L], f32, kind="ExternalInput")
    cw = nc.dram_tensor("cw", [HS, 2 * H], f32, kind="ExternalInput")
    cb = nc.dram_tensor("cb", [HS], f32, kind="ExternalInput")
    wihC = nc.dram_tensor("wihC", [GR, HS], f32, kind="ExternalInput")
    whhC = nc.dram_tensor("whhC", [GR, HS], f32, kind="ExternalInput")
    h0row = nc.dram_tensor("h0row", [1, H], f32, kind="ExternalInput")
    h0k = nc.dram_tensor("h0k", [HS], f32, kind="ExternalInput")

    arg_o = nc.dram_tensor("arg", [2 * GR], f32, kind="ExternalOutput")
    attn_o = nc.dram_tensor("attn", [1, L], f32, kind="ExternalOutput")

    areg = nc.dram_tensor("areg", [L], f32)
    creg = nc.dram_tensor("creg", [H], f32)
    xreg = nc.dram_tensor("xreg", [HS], f32)

    def bc(handle, n, cnt):
        a = handle.ap()
        return bass.AP(tensor=a.tensor, offset=a.offset, ap=[[0, n], [1, cnt]])

    NW = GR // 1024  # 6 wide tiles per gate-weight matrix

    with tile.TileContext(nc) as tc:
        with (
            tc.tile_pool(name="singles", bufs=1) as sg,
            tc.tile_pool(name="gw", bufs=4) as gwp,
            tc.tile_pool(name="scr", bufs=3) as scrp,
        ):
            eb = sg.tile([128, H], f32)
            nc.sync.dma_start(out=eb, in_=bc(erow, 128, H))
            h0b = sg.tile([128, H], f32)
            nc.sync.dma_start(out=h0b, in_=bc(h0row, 128, H))
            # h0 contraction-slice broadcast for the gh partials (ready at t=0)
            hkb = sg.tile([128, HS], f32)
            nc.sync.dma_start(out=hkb, in_=bc(h0k, 128, HS))

            # ---- partial gh: w_hh[:, k-cols] rows on partitions ----
            arg_sb = sg.tile([128, 2 * GR // 128], f32)   # [128, 96] lanes j=...
            for jt in range(NW):
                w_t = gwp.tile([128, 8, HS], f32, name=f"whc{jt}", tag="gw")
                nc.sync.dma_start(
                    out=w_t,
                    in_=whhC[1024 * jt : 1024 * (jt + 1), :].rearrange(
                        "(p g) c -> p g c", p=128
                    ),
                )
                s_t = scrp.tile([128, 8, HS], f32, name=f"shc{jt}", tag="scr")
                hk_ap = hkb[:, :]
                hk3 = bass.AP(tensor=hk_ap.tensor, offset=hk_ap.offset,
                              ap=[hk_ap.ap[0], [0, 8], hk_ap.ap[1]])
                nc.vector.tensor_mul(s_t, w_t, hk3)
                if jt % 2 == 0:
                    nc.vector.reduce_sum(
                        arg_sb[:, 48 + 8 * jt : 48 + 8 * jt + 8], s_t, axis=X
                    )
                else:
                    for g in range(8):
                        nc.scalar.activation(
                            s_t[:, g, :], s_t[:, g, :], AF.Identity,
                            accum_out=arg_sb[:, 48 + 8 * jt + g : 48 + 8 * jt + g + 1],
                        )

            # ---------- attention ----------
            adots = sg.tile([L, 2], f32)
            for hh in range(2):
                awh = gwp.tile([L, H], f32, name=f"awh{hh}", tag="gw")
                nc.sync.dma_start(out=awh, in_=aw[:, hh * H : (hh + 1) * H])
                s_t = scrp.tile([L, H], f32, name=f"ascr{hh}", tag="scr")
                nc.vector.tensor_mul(s_t, awh, (eb if hh == 0 else h0b)[:L, :])
                nc.scalar.activation(s_t, s_t, AF.Identity,
                                     accum_out=adots[:, hh : hh + 1])
            acol = sg.tile([L, 1], f32)
            nc.vector.tensor_add(acol, adots[:, 0:1], adots[:, 1:2])
            arow = sg.tile([1, L], f32)
            nc.sync.dma_start(out=arow, in_=acol)
            ab_sb = sg.tile([1, L], f32)
            nc.sync.dma_start(out=ab_sb, in_=ab[:, :])
            nc.vector.tensor_add(arow, arow, ab_sb)
            amax = sg.tile([1, 1], f32)
            nc.vector.reduce_max(amax, arow, axis=X)
            negm = sg.tile([1, 1], f32)
            nc.vector.tensor_scalar_mul(negm, amax, -1.0)
            asum = sg.tile([1, 1], f32)
            nc.scalar.activation(arow, arow, AF.Exp, bias=negm, scale=1.0,
                                 accum_out=asum)
            rcp = sg.tile([1, 1], f32)
            nc.vector.reciprocal(rcp, asum)
            nc.vector.tensor_scalar_mul(arow, arow, rcp)
            nc.sync.dma_start(out=attn_o[:, :], in_=arow)
            nc.sync.dma_start(out=areg.ap(), in_=arow)

            # ---------- attn_applied (partition-major creg layout) ----------
            encT_sb = sg.tile([128, H // 128, L], f32)
            nc.sync.dma_start(
                out=encT_sb, in_=encT.ap().rearrange("(i p) j -> p i j", p=128)
            )
            awb = sg.tile([128, L], f32)
            nc.sync.dma_start(out=awb, in_=bc(areg, 128, L))
            awb_ap = awb[:, :]
            awb3 = bass.AP(tensor=awb_ap.tensor, offset=awb_ap.offset,
                           ap=[awb_ap.ap[0], [0, H // 128], awb_ap.ap[1]])
            ctxp = sg.tile([128, H // 128, L], f32)
            nc.vector.tensor_mul(ctxp, encT_sb, awb3)
            ctxcol = sg.tile([128, H // 128], f32)
            nc.vector.reduce_sum(ctxcol, ctxp, axis=X)
            nc.sync.dma_start(
                out=creg.ap().rearrange("(p i) -> p i", p=128), in_=ctxcol
            )
            crb = sg.tile([128, H], f32)
            nc.sync.dma_start(out=crb, in_=bc(creg, 128, H))

            # ---------- comb matvec -> x_k ----------
            cb_sb = sg.tile([128, 2], f32)
            nc.sync.dma_start(out=cb_sb, in_=cb.ap().rearrange("(m p) -> p m", p=128))
            xq = sg.tile([128, 4], f32)
            for hh in range(2):
                for m in range(2):
                    w_q = gwp.tile([128, H], f32, name=f"wq{hh}{m}", tag="gw")
                    nc.sync.dma_start(
                        out=w_q, in_=cw[m * 128 : (m + 1) * 128, hh * H : (hh + 1) * H]
                    )
                    s_t = scrp.tile([128, H], f32, name=f"cscr{hh}{m}", tag="scr")
                    nc.vector.tensor_mul(s_t, w_q, eb if hh == 0 else crb)
                    nc.scalar.activation(s_t, s_t, AF.Identity,
                                         accum_out=xq[:, hh * 2 + m : hh * 2 + m + 1])
            xpre = sg.tile([128, 2], f32)
            nc.vector.tensor_add(xpre, xq[:, 0:2], xq[:, 2:4])
            x_sb = sg.tile([128, 2], f32)
            for m in range(2):
                nc.scalar.activation(
                    x_sb[:, m : m + 1], xpre[:, m : m + 1], AF.Relu,
                    bias=cb_sb[:, m : m + 1], scale=1.0,
                )
            nc.sync.dma_start(
                out=xreg.ap().rearrange("(p m) -> p m", p=128), in_=x_sb
            )
            xkb = sg.tile([128, HS], f32)
            nc.sync.dma_start(out=xkb, in_=bc(xreg, 128, HS))

            # ---- partial gi: w_ih[:, k-cols] ----
            for jt in range(NW):
                w_t = gwp.tile([128, 8, HS], f32, name=f"wic{jt}", tag="gw")
                nc.sync.dma_start(
                    out=w_t,
                    in_=wihC[1024 * jt : 1024 * (jt + 1), :].rearrange(
                        "(p g) c -> p g c", p=128
                    ),
                )
                s_t = scrp.tile([128, 8, HS], f32, name=f"sic{jt}", tag="scr")
                xk_ap = xkb[:, :]
                xk3 = bass.AP(tensor=xk_ap.tensor, offset=xk_ap.offset,
                              ap=[xk_ap.ap[0], [0, 8], xk_ap.ap[1]])
                nc.vector.tensor_mul(s_t, w_t, xk3)
                if jt % 2 == 1:
                    nc.vector.reduce_sum(
                        arg_sb[:, 8 * jt : 8 * jt + 8], s_t, axis=X
                    )
                else:
                    for g in range(8):
                        nc.scalar.activation(
                            s_t[:, g, :], s_t[:, g, :], AF.Identity,
                            accum_out=arg_sb[:, 8 * jt + g : 8 * jt + g + 1],
                        )

            # store partials partition-major; host reorders to natural j
            nc.sync.dma_start(
                out=arg_o.ap().rearrange("(p col) -> p col", p=128), in_=arg_sb
            )

    nc.compile()
    return nc


def _build_k2():
    import concourse.bass as bass
    import concourse.bacc as bacc
    import concourse.tile as tile
    from concourse import mybir

    f32 = mybir.dt.float32
    AF = mybir.ActivationFunctionType
    X = mybir.AxisListType.X

    nc = bacc.Bacc("TRN2", target_bir_lowering=False, debug=False, num_devices=NC)

    argsum = nc.dram_tensor("argsum", [2 * GR], f32, kind="ExternalInput")
    bih = nc.dram_tensor("bih", [GR], f32, kind="ExternalInput")
    bhh = nc.dram_tensor("bhh", [GR], f32, kind="ExternalInput")
    h0v = nc.dram_tensor("h0v", [H], f32, kind="ExternalInput")
    ow = nc.dram_tensor("ow", [VS, H], f32, kind="ExternalInput")
    ob = nc.dram_tensor("ob", [VPAD], f32, kind="ExternalInput")

    logits_o = nc.dram_tensor("logits", [128, VT], f32, kind="ExternalOutput")
    hnew_o = nc.dram_tensor("hnew", [1, H], f32, kind="ExternalOutput")

    hreg = nc.dram_tensor("hreg", [H], f32)

    def bc(handle, n, cnt):
        a = handle.ap()
        return bass.AP(tensor=a.tensor, offset=a.offset, ap=[[0, n], [1, cnt]])

    def lanes(handle, base):
        # [128, 16] view of handle[base : base+2048], lane (p,c) = elem 16p+c
        a = handle.ap()
        return bass.AP(tensor=a.tensor, offset=a.offset + base,
                       ap=[[16, 128], [1, 16]])

    with tile.TileContext(nc) as tc:
        with (
            tc.tile_pool(name="singles", bufs=1) as sg,
            tc.tile_pool(name="lw", bufs=7) as lwp,
            tc.tile_pool(name="scr", bufs=4) as scrp,
            tc.tile_pool(name="gscr", bufs=2) as gscrp,
        ):
            # ---------- gates + h_new (lanes j = 16p + c) ----------
            gi = [sg.tile([128, 16], f32, name=f"gi{b}") for b in range(3)]
            gh = [sg.tile([128, 16], f32, name=f"gh{b}") for b in range(3)]
            for b in range(3):
                nc.sync.dma_start(out=gi[b], in_=lanes(argsum, b * H))
                nc.sync.dma_start(out=gh[b], in_=lanes(argsum, GR + b * H))
            bi = [sg.tile([128, 16], f32, name=f"bi{b}") for b in range(3)]
            bh = [sg.tile([128, 16], f32, name=f"bh{b}") for b in range(3)]
            for b in range(3):
                nc.sync.dma_start(out=bi[b], in_=lanes(bih, b * H))
                nc.sync.dma_start(out=bh[b], in_=lanes(bhh, b * H))
            h0t = sg.tile([128, 16], f32)
            nc.sync.dma_start(out=h0t, in_=lanes(h0v, 0))

            for b in range(3):
                nc.vector.tensor_add(gi[b], gi[b], bi[b])
                nc.vector.tensor_add(gh[b], gh[b], bh[b])
            # r, z
            rt = sg.tile([128, 16], f32)
            nc.vector.tensor_add(rt, gi[0], gh[0])
            r_sb = sg.tile([128, 16], f32)
            nc.scalar.activation(r_sb, rt, AF.Sigmoid)
            zt = sg.tile([128, 16], f32)
            nc.vector.tensor_add(zt, gi[1], gh[1])
            z_sb = sg.tile([128, 16], f32)
            nc.scalar.activation(z_sb, zt, AF.Sigmoid)
            # n
            nt = sg.tile([128, 16], f32)
            nc.vector.tensor_mul(nt, r_sb, gh[2])
            nc.vector.tensor_add(nt, nt, gi[2])
            n_sb = sg.tile([128, 16], f32)
            nc.scalar.activation(n_sb, nt, AF.Tanh)
            # h' = n + z*(h0 - n)
            d_sb = sg.tile([128, 16], f32)
            nc.vector.tensor_sub(d_sb, h0t, n_sb)
            nc.vector.tensor_mul(d_sb, z_sb, d_sb)
            hn_sb = sg.tile([128, 16], f32)
            nc.vector.tensor_add(hn_sb, n_sb, d_sb)
            nc.sync.dma_start(
                out=hreg.ap().rearrange("(p c) -> p c", p=128), in_=hn_sb
            )
            nc.sync.dma_start(
                out=hnew_o[:, :], in_=hreg.ap().rearrange("(a f) -> a f", a=1)
            )
            hb = sg.tile([128, H], f32)
            nc.sync.dma_start(out=hb, in_=bc(hreg, 128, H))

            # ---------- logits ----------
            ob_sb = sg.tile([128, VT], f32)
            nc.sync.dma_start(
                out=ob_sb, in_=ob.ap().rearrange("(t p) -> p t", p=128)
            )
            logit_sb = sg.tile([128, VT], f32)
            nc.vector.memset(logit_sb, 0.0)
            for tt in range(VT // 2):
                w2 = lwp.tile([128, 2, H], f32, name=f"lw{tt}", tag="lw")
                r2 = min(256, VS - tt * 256)
                dma_eng = nc.sync
                if r2 == 256:
                    dma_eng.dma_start(
                        out=w2,
                        in_=ow[tt * 256 : (tt + 1) * 256, :].rearrange(
                            "(g p) f -> p g f", p=128
                        ),
                    )
                else:
                    dma_eng.dma_start(
                        out=w2[:, 0, :], in_=ow[tt * 256 : tt * 256 + 128, :]
                    )
                    dma_eng.dma_start(
                        out=w2[: r2 - 128, 1, :], in_=ow[tt * 256 + 128 : VS, :]
                    )
                for g in range(2):
                    t = 2 * tt + g
                    rows = min(128, VS - t * 128)
                    on_gps = (t % 4 == 3) or t == 0
                    if on_gps:
                        s_t = gscrp.tile([128, H], f32, name=f"gls{t}", tag="gscr")
                        nc.gpsimd.tensor_mul(
                            s_t[:rows, :], w2[:rows, g, :], hb[:rows, :]
                        )
                    else:
                        s_t = scrp.tile([128, H], f32, name=f"ls{t}", tag="scr")
                        nc.vector.tensor_mul(
                            s_t[:rows, :], w2[:rows, g, :], hb[:rows, :]
                        )
                    if t in (1, 9, 17, 25, 33, 41):
                        nc.vector.reduce_sum(
                            logit_sb[:rows, t : t + 1], s_t[:rows, :], axis=X
                        )
                    else:
                        nc.scalar.activation(
                            s_t[:rows, :], s_t[:rows, :], AF.Identity,
                            accum_out=logit_sb[:rows, t : t + 1],
                        )
            nc.vector.tensor_add(logit_sb, logit_sb, ob_sb)
            nc.sync.dma_start(out=logits_o[:, :], in_=logit_sb)

    nc.compile()
    return nc


def _marshal_k1(input_ids, hidden, encoder_outputs, emb, attn_w, attn_b,
                comb_w, comb_b, w_ih, w_hh, out_w=None, out_b=None,
                b_ih=None, b_hh=None):
    f = np.float32
    ii = int(np.asarray(input_ids).ravel()[0])
    erow = np.ascontiguousarray(np.asarray(emb)[ii], dtype=f).reshape(1, H)
    h0 = np.ascontiguousarray(np.asarray(hidden, f).reshape(H))
    common = {
        "erow": erow,
        "aw": np.asarray(attn_w, f),
        "ab": np.asarray(attn_b, f).reshape(1, L),
        "encT": np.ascontiguousarray(np.asarray(encoder_outputs, f).T),
        "h0row": h0.reshape(1, H),
    }
    cwf = np.asarray(comb_w, f)
    cbf = np.asarray(comb_b, f)
    wihf = np.asarray(w_ih, f)
    whhf = np.asarray(w_hh, f)
    in_maps = []
    for k in range(NC):
        r0 = HS * k
        m = dict(common)
        m["cw"] = np.concatenate(
            [cwf[r0 : r0 + HS, :H], cwf[r0 : r0 + HS, H:][:, _CREG_PERM]], axis=1
        )
        m["cb"] = cbf[r0 : r0 + HS]
        m["wihC"] = np.ascontiguousarray(wihf[:, r0 : r0 + HS][:, _XREG_PERM])
        m["whhC"] = np.ascontiguousarray(whhf[:, r0 : r0 + HS])
        m["h0k"] = h0[r0 : r0 + HS]
        in_maps.append(m)
    return in_maps


def _marshal_k2(argsum, hidden, b_ih, b_hh, out_w, out_b):
    f = np.float32
    h0 = np.ascontiguousarray(np.asarray(hidden, f).reshape(H))
    owf = np.asarray(out_w, f)
    obf = np.asarray(out_b, f)
    common = {
        "argsum": argsum,
        "bih": np.asarray(b_ih, f),
        "bhh": np.asarray(b_hh, f),
        "h0v": h0,
    }
    in_maps = []
    for k in range(NC):
        v0 = VS * k if k < NC - 1 else V - VS
        obk = np.zeros(VPAD, f)
        obk[:VS] = obf[v0 : v0 + VS]
        m = dict(common)
        m["ow"] = owf[v0 : v0 + VS]
        m["ob"] = obk
        in_maps.append(m)
    return in_maps


def kernel(**inputs):
    global LAST_RESULT
    from concourse.bass_utils import run_bass_kernel_spmd

    if "k1" not in _NC_CACHE:
        _NC_CACHE["k1"] = _build_k1()
        _NC_CACHE["k2"] = _build_k2()
    nc1, nc2 = _NC_CACHE["k1"], _NC_CACHE["k2"]

    kwargs = {}
    if TRACE:
        import concourse.bass_utils as bu
        bu.upload_artifacts = lambda d: str(d)
        kwargs = dict(trace=True, trace_cores=[0])

    in1 = _marshal_k1(
        inputs["input_ids"], inputs["hidden"], inputs["encoder_outputs"],
        inputs["emb"], inputs["attn_w"], inputs["attn_b"],
        inputs["comb_w"], inputs["comb_b"], inputs["w_ih"], inputs["w_hh"],
    )
    res1 = run_bass_kernel_spmd(nc1, in1, core_ids=list(range(NC)), **kwargs)

    arg_p = np.zeros(2 * GR, np.float32)
    for k in range(NC):
        arg_p += res1.results[k]["arg"].reshape(-1)
    # device layout [p, half, jt, g] -> natural j = jt*1024 + 8p + g per half
    argsum = np.ascontiguousarray(
        arg_p.reshape(128, 2, GR // 1024, 8).transpose(1, 2, 0, 3).reshape(-1)
    )

    in2 = _marshal_k2(argsum, inputs["hidden"], inputs["b_ih"], inputs["b_hh"],
                      inputs["out_w"], inputs["out_b"])
    res2 = run_bass_kernel_spmd(nc2, in2, core_ids=list(range(NC)), **kwargs)
    LAST_RESULT = (res1, res2)

    logits = np.empty((1, V), np.float32)
    for k in range(NC):
        v0 = VS * k if k < NC - 1 else V - VS
        arr = res2.results[k]["logits"]          # [128, VT]
        logits[0, v0 : v0 + VS] = arr.T.reshape(-1)[:VS]
    hnew = res2.results[0]["hnew"].reshape(1, 1, H).astype(np.float32)
    attn = res1.results[0]["attn"].reshape(1, L).astype(np.float32)
    return logits, hnew, attn
